# revision 32
# baseline (speedup 1.0000x reference)
"""Trainium2 Bass kernel for nn_LocalAggregator (GNN message passing).

Math (per batch):
    e[i,j,r] = lrelu( h_i . diag(a_r) . h_j  +  sum_t cos(A_ij f_t + p_t) iw[t,r] )
    s[i,j]   = e[i,j,adj_ij-1]  if 1<=adj<=5 else -9e15
    out      = softmax_j(s) @ h

Device strategy (per core, BL=4 of the 32 batches), v2:
  * Everything score-shaped lives TRANSPOSED: sT[j, (b, i)] — legal because
    e1_c = H diag(a_c) H^T is symmetric, and A/adj are transposed host-side.
    This kills the per-batch PE transposes and PSUM->SBUF copies: exp(s)^T
    is directly the stationary operand of the output matmul, and the row
    sums Z come free from a ones-column appended to the moving operand h.
  * e1: bf16 matmuls (1 cyc/row vs fp32's 4) with the 5 classes PACKED into
    the moving operand — hTa5[d,(ch,b,c,i)] host-prescaled by a_c. Per
    (ch,b): one 512-row matmul (classes 0-3 -> psA) + one 128-row (class 4
    -> psB). 16 matmuls / 8 stationary loads instead of 40 / 40.
  * Time-encoding branch g_c(A): host fits a degree-DEG polynomial per
    class (deg-4 max fit err ~5e-3) and GATHERS per-element coefficient
    maps cm_k[j,(b,i)] = C[k, adj-1] — the class select for the polynomial
    happens on the host for free. The device runs ONE Horner chain with
    plain tensor_tensor ops (2(DEG-1) DVE ops total vs 5 per-class chains).
    adj==0 poisoning: cm_0 = -1e5 there (and higher coeffs 0), so after
    lrelu and exp those entries vanish — no separate mask op.
  * e1 class select: scalar-engine copy (class 0 base) + 4 DVE
    copy_predicated with host int8 masks; then one DVE add folds the
    polynomial in, one DVE scalar_tensor_tensor does the leaky relu.
  * exp on the scalar engine -> bf16; out matmul [exp^T stationary] x
    [h | 1] moving gives output AND row-sum Z; 1/Z folds into the
    PSUM->SBUF scaled copy; output DMA'd in bf16.
  * Two walrus version-skew workarounds: the Tile tail drain and any
    instruction may carry at most ONE sync-wait command on this toolchain
    (_patch_tail_drain / _split_excess_waits hoist excess waits onto NoOps).
"""

import os
from contextlib import ExitStack

import numpy as np
import ml_dtypes

BF16 = ml_dtypes.bfloat16

B, N, D, TDIM = 32, 128, 256, 64
NCORES = 8
BL = B // NCORES            # batches per core
ALPHA = 0.2
NEG_INF = -9e15
POISON = -5e4               # adj==0 polynomial poison (f16-finite; exp() == 0)
DEG = 4                     # host-fitted polynomial degree (DEG+1 coefficients)
NWARM = 24                  # PE warm-up matmuls issued during the DMA window
DCH = D // 128              # K-chunks for the e1 contraction
FBJ = BL * N                # 512: free size of (b, i)

_PROG_CACHE: dict = {}
_DRAIN_PATCHED = False


def _patch_tail_drain():
    """Version-skew workaround: the TileContext tail drain accumulates one
    sem-wait per outstanding engine/DMA queue, but this walrus build's Drain
    encoding fits only ONE sync-wait command. Spread the excess waits over
    preceding single-wait NoOps on the same (SP) engine."""
    global _DRAIN_PATCHED
    if _DRAIN_PATCHED:
        return
    import concourse.tile as tile_mod

    def _patched(self, tick_clock, wait_clock):
        nc = self.nc
        drain_inst = nc.sync.drain()
        wait_clock.add_sem_waits(
            drain_inst.ins,
            tile_mod.ScopedClock({None: tick_clock.global_clock}),
        )
        mi = drain_inst.ins
        si = mi.sync_info
        waits = list(si.on_wait) if si is not None and si.on_wait else []
        if len(waits) > 1:
            si.on_wait = waits[:1]
            lst = nc.cur_bb.bb.instructions
            assert lst[-1] is mi, "drain is not the last instruction in block"
            drain_obj = lst.pop()
            for w in waits[1:]:
                nop = nc.sync.nop(nofuse=True)
                nsi = nop.ins.sync_info
                if nsi is None:
                    nop.ins.sync_info = type(si)(on_update=[], on_wait=[w])
                else:
                    nsi.on_wait = [w]
            lst.append(drain_obj)
        nc.all_engine_barrier()
        assert self.sems is not None
        popped = nc._tile_sem_poison_stack.pop()
        assert popped is self._sem_poison
        nc.clear_and_free_semaphores(list(self.sems.allocated().values()))

    tile_mod.TileContext._drain_and_barrier = _patched
    _DRAIN_PATCHED = True


def _split_excess_waits(nc, max_waits: int = 1):
    """This walrus build encodes at most one sync-wait command per
    instruction. Hoist excess waits onto same-engine NoOps inserted
    immediately before the over-subscribed instruction."""
    import concourse.mybir as mybir

    for fn in nc.m.functions:
        for bb in fn.blocks:
            insts = bb.instructions
            i = 0
            while i < len(insts):
                inst = insts[i]
                si = getattr(inst, "sync_info", None)
                waits = list(si.on_wait) if si is not None and si.on_wait else []
                if len(waits) > max_waits:
                    si.on_wait = waits[:max_waits]
                    extra = waits[max_waits:]
                    nops = []
                    for k in range(0, len(extra), max_waits):
                        nops.append(
                            mybir.InstNoOp(
                                name=f"{inst.name}-xw{k}",
                                engine=inst.engine,
                                bass_nofuse=True,
                                sync_info=mybir.SyncInfo(
                                    on_wait=extra[k : k + max_waits], on_update=[]
                                ),
                            )
                        )
                    insts[i:i] = nops
                    i += len(nops)
                i += 1


# --------------------------------------------------------------------------
# host-side parameter preprocessing
# --------------------------------------------------------------------------
def _fit_polys(iw_params: np.ndarray, te_freq: np.ndarray, te_phase: np.ndarray):
    """Least-squares fit of g_c(a) = sum_t iw[t,c] cos(a f_t + p_t), a in [0,1].

    Returns C[k, c] for k=0..DEG, in the CENTERED basis t = a - 0.5
    (keeps coefficients small enough that f16 maps round harmlessly).
    """
    npts = 1024
    x = 0.5 * (1.0 + np.cos(np.pi * (np.arange(npts) + 0.5) / npts))
    f = te_freq.astype(np.float64)
    p = te_phase.astype(np.float64)
    iw = iw_params.astype(np.float64)
    G = np.cos(x[:, None] * f[None, :] + p[None, :]) @ iw      # (npts, 5)
    V = np.vander(x - 0.5, DEG + 1, increasing=True)           # (npts, DEG+1)
    C, *_ = np.linalg.lstsq(V, G, rcond=None)
    return C  # (DEG+1, 5) float64


# --------------------------------------------------------------------------
# Bass program
# --------------------------------------------------------------------------
def _build_program():
    """One-core program; SPMD across 8 cores with per-core input maps.

    All score-shaped tiles are TRANSPOSED: [j, (b, i)] with j (the softmax
    axis) on partitions. The polynomial coefficients arrive as per-element
    maps, so the program itself is independent of the fitted coefficients.
    """
    import concourse.bass as bass
    import concourse.mybir as mybir
    import concourse.tile as tile

    _patch_tail_drain()

    f32 = mybir.dt.float32
    bf16 = mybir.dt.bfloat16
    f16 = mybir.dt.float16
    i8 = mybir.dt.int8
    i16 = mybir.dt.int16
    Alu = mybir.AluOpType
    Act = mybir.ActivationFunctionType

    nc = bass.Bass()

    # DRAM I/O (per-core layouts; host arranges)
    xT_d = nc.dram_tensor("xT", [N, FBJ], f16, kind="ExternalInput")        # (A^T - 0.5) [j,(b,i)]
    cm_d = nc.dram_tensor("cm", [N, (DEG + 1) * FBJ], f16,
                          kind="ExternalInput")                             # coeff maps k=DEG..0
    m14_d = nc.dram_tensor("m14", [N, 4 * FBJ], i8, kind="ExternalInput")   # masks cls 1..4
    hT_d = nc.dram_tensor("hT", [128, DCH * BL * 128], bf16,
                          kind="ExternalInput")                             # [d,(ch,b,j)]
    ap_d = nc.dram_tensor("ap", [128, DCH * 5], f32, kind="ExternalInput")  # a_params [d,(ch,c)]
    haug_d = nc.dram_tensor("haug", [N, BL * (D + 1)], bf16,
                            kind="ExternalInput")                           # [j,(b,d|1)]
    out_d = nc.dram_tensor("out", [N, BL * D], bf16, kind="ExternalOutput") # [i,(b,d)]

    CH_STRIDE = 5 * BL * 128   # 2560: hTa5 per-chunk stride (c-major inside)

    with tile.TileContext(nc) as tc, ExitStack() as ctx:
        io = ctx.enter_context(tc.tile_pool(name="io", bufs=1))
        wrk = ctx.enter_context(tc.tile_pool(name="wrk", bufs=1))

        # ---- PE warm-up scratch: memset issues before any DMA ----
        warm = wrk.tile([128, 128], bf16, tag="warm")
        nc.gpsimd.memset(warm[:], 0)

        # ---- loads, spread across the 3 DMA queues so the gating tensors
        # (ap/hT for the matmul path, xT/cm for the DVE path) land first ----
        ap = io.tile([128, DCH * 5], f32, tag="ap")
        nc.scalar.dma_start(ap[:], ap_d[:])
        xT = io.tile([N, FBJ], f16, tag="xT")
        nc.scalar.dma_start(xT[:], xT_d[:])
        cm = io.tile([N, (DEG + 1) * FBJ], f16, tag="cm")
        nc.gpsimd.dma_start(cm[:, 0:FBJ], cm_d[:, 0:FBJ])
        hT = io.tile([128, DCH * BL * 128], bf16, tag="hT")
        nc.sync.dma_start(hT[:, 0:BL * 128], hT_d[:, 0:BL * 128])
        nc.sync.dma_start(hT[:, BL * 128:], hT_d[:, BL * 128:])
        nc.gpsimd.dma_start(cm[:, FBJ:2 * FBJ], cm_d[:, FBJ:2 * FBJ])
        nc.sync.dma_start(cm[:, 2 * FBJ:3 * FBJ], cm_d[:, 2 * FBJ:3 * FBJ])
        nc.scalar.dma_start(cm[:, 3 * FBJ:], cm_d[:, 3 * FBJ:])
        m14 = io.tile([N, 4 * FBJ], i8, tag="m14")
        nc.gpsimd.dma_start(m14[:], m14_d[:])
        haug = io.tile([N, BL * (D + 1)], bf16, tag="haug")
        nc.gpsimd.dma_start(haug[:], haug_d[:])

        # ---- DVE front half, interleaved with DMA arrivals: scale hT by
        # a_c (bf16 tensor_scalar) around the f16 Horner chain so the DVE
        # never stalls on a single DMA's completion semaphore ----
        # hTa5 layout: [d, (ch, c, b, i)]
        hTa5 = wrk.tile([128, DCH * CH_STRIDE], bf16, tag="hTa5")

        def scale_chunk(ch):
            # class-4 scale runs on the otherwise-idle GPSIMD engine; its
            # psB matmuls are ordered last per chunk to hide the latency
            for c in range(5):
                eng = nc.gpsimd if c == 4 else nc.vector
                eng.tensor_scalar(
                    hTa5[:, ch * CH_STRIDE + c * 512:
                         ch * CH_STRIDE + (c + 1) * 512],
                    hT[:, ch * 512:(ch + 1) * 512],
                    ap[:, ch * 5 + c: ch * 5 + c + 1], None, Alu.mult)

        u = wrk.tile([N, FBJ], f16, tag="u")
        scale_chunk(0)
        scale_chunk(1)
        # Horner with per-element coeff maps; cm layout k = DEG ... 0
        nc.vector.tensor_tensor(u[:], cm[:, 0:FBJ], xT[:], Alu.mult)
        for k in range(1, DEG):
            nc.vector.tensor_tensor(u[:], u[:], cm[:, k * FBJ:(k + 1) * FBJ],
                                    Alu.add)
            nc.vector.tensor_tensor(u[:], u[:], xT[:], Alu.mult)
        nc.vector.tensor_tensor(u[:], u[:], cm[:, DEG * FBJ:(DEG + 1) * FBJ],
                                Alu.add)

        # ---- e1 matmuls (classes packed in the moving operand) ----
        s = wrk.tile([N, FBJ], f32, tag="s")
        with tc.tile_pool(name="psum", bufs=1, space="PSUM") as psum:
            psA = psum.tile([N, BL * 512], f32, tag="psA", name="psA")  # (b, c0..3, i)
            psB = psum.tile([N, BL * 128], f32, tag="psB", name="psB")  # (b, c4, i)
            pwarm = psum.tile([128, 128], f32, tag="pwarm", name="pwarm")
            for w in range(NWARM):
                nc.tensor.matmul(pwarm[:], warm[:], warm[:],
                                 start=True, stop=True, skip_group_check=True)
            # hTa5 free layout per chunk is (c, b, i); for batch b the
            # moving operand is the strided 2D view (c x i) at that b.
            hTa5_v = hTa5[:].rearrange("p (ch c b i) -> p ch c b i",
                                       ch=DCH, c=5, b=BL, i=128)
            for ch in range(DCH):
                for b in range(BL):
                    stat = hT[:, (ch * BL + b) * 128:(ch * BL + b + 1) * 128]
                    nc.tensor.matmul(
                        psA[:, b * 512:(b + 1) * 512], stat,
                        hTa5_v[:, ch, 0:4, b, :],
                        start=(ch == 0), stop=(ch == DCH - 1),
                        skip_group_check=True)
                for b in range(BL):
                    stat = hT[:, (ch * BL + b) * 128:(ch * BL + b + 1) * 128]
                    nc.tensor.matmul(
                        psB[:, b * 128:(b + 1) * 128], stat,
                        hTa5_v[:, ch, 4, b, :],
                        start=(ch == 0), stop=(ch == DCH - 1),
                        skip_group_check=True)

            # ---- select e1 by adj class, HALF-SPLIT (b0,b1 | b2,b3) so the
            # whole back end pipelines. The scalar engine (idle here) stages
            # each class map PSUM->SBUF as f16 per half; DVE then applies
            # the predicated class merges per half. ----
            psA_v = psA[:].rearrange("p (b c i) -> p b c i", b=BL, c=4, i=128)
            sf = wrk.tile([N, FBJ], f16, tag="sf")
            sf_v = sf[:].rearrange("p (b i) -> p b i", b=BL, i=128)
            e14 = wrk.tile([N, 4 * FBJ], f16, tag="e14")
            e14_v = e14[:].rearrange("p (c b i) -> p c b i", c=4, b=BL, i=128)
            H = FBJ // 2
            nc.scalar.copy(sf_v, psA_v[:, :, 0, :])       # base: class 0
            for c in range(1, 4):
                nc.scalar.copy(e14_v[:, c - 1, :, :], psA_v[:, :, c, :])
            nc.scalar.copy(e14[:, 3 * FBJ:4 * FBJ], psB[:])

        # per half: class merges, fold polynomial, leaky relu (DVE)
        for h in range(2):
            hs = slice(h * H, (h + 1) * H)
            for c in range(1, 5):
                nc.vector.copy_predicated(
                    sf[:, hs], m14[:, (c - 1) * FBJ + h * H:
                                   (c - 1) * FBJ + (h + 1) * H],
                    e14[:, (c - 1) * FBJ + h * H:(c - 1) * FBJ + (h + 1) * H])
            nc.vector.tensor_tensor(s[:, hs], sf[:, hs], u[:, hs], Alu.add)
            nc.vector.scalar_tensor_tensor(
                s[:, hs], s[:, hs], ALPHA, s[:, hs], Alu.mult, Alu.max)

        # ---- exp -> bf16; out matmul with ones-column Z; scaled copy ----
        ex = wrk.tile([N, FBJ], bf16, tag="ex")
        rz = wrk.tile([N, BL], f32, tag="rz")
        out_sb = wrk.tile([N, BL * D], bf16, tag="out")
        with tc.tile_pool(name="psum2", bufs=1, space="PSUM") as psum2:
            ops = [psum2.tile([N, D + 1], f32, tag=f"op_{b}", name=f"op_{b}")
                   for b in range(BL)]
            for h in range(2):
                hs = slice(h * H, (h + 1) * H)
                nc.scalar.activation(ex[:, hs], s[:, hs], Act.Exp, bias=0.0)
                for b in (2 * h, 2 * h + 1):
                    nc.tensor.matmul(
                        ops[b][:],
                        ex[:, b * 128:(b + 1) * 128],
                        haug[:, b * (D + 1):(b + 1) * (D + 1)],
                    )
                    nc.vector.reciprocal(rz[:, b:b + 1], ops[b][:, D:D + 1])
                    # normalize: alternate DVE / scalar so the tail overlaps
                    if b % 2 == 0:
                        nc.vector.tensor_scalar(
                            out_sb[:, b * D:(b + 1) * D], ops[b][:, 0:D],
                            rz[:, b:b + 1], None, Alu.mult)
                    else:
                        nc.scalar.mul(out_sb[:, b * D:(b + 1) * D],
                                      ops[b][:, 0:D], rz[:, b:b + 1])
                    # DVE-normalized batches issue on the scalar queue and
                    # scalar-normalized ones on sync, so issue never waits
                    # behind the engine that just produced the data
                    eng = nc.scalar if b % 2 == 0 else nc.sync
                    eng.dma_start(out_d[:, b * D:(b + 1) * D],
                                  out_sb[:, b * D:(b + 1) * D])

    return nc


# --------------------------------------------------------------------------
# host-side input maps
# --------------------------------------------------------------------------
def build_in_maps(inputs: dict) -> list:
    hidden = np.ascontiguousarray(inputs["hidden"], dtype=np.float32)   # (B,N,D)
    A = np.ascontiguousarray(inputs["A_interval"], dtype=np.float32)    # (B,N,N)
    adj = np.asarray(inputs["adj"])                                     # (B,N,N) i32
    a_params = np.asarray(inputs["a_params"], dtype=np.float32)         # (D,5)
    Cpoly = _fit_polys(np.asarray(inputs["iw_params"]),
                       np.asarray(inputs["te_freq"]),
                       np.asarray(inputs["te_phase"]))                  # (DEG+1, 5)

    # coefficient lookup tables indexed by adj value 0..5 (0 = poisoned)
    # order in cm: k = DEG, DEG-1, ..., 0
    lut = np.zeros((DEG + 1, 6), np.float16)
    for k in range(DEG + 1):
        lut[DEG - k, 1:] = Cpoly[k, :]          # adj v -> class v-1
    lut[DEG, 0] = POISON                        # c0 for adj==0

    in_maps = []
    for core in range(NCORES):
        bs = slice(core * BL, (core + 1) * BL)
        hs = hidden[bs]                                   # (BL,N,D)
        adjs = adj[bs]
        assert ((adjs >= 1) & (adjs <= 5)).any(axis=2).all(), (
            "row with no valid edge: shift-free softmax unsupported")

        # xT: [j, (b, i)] = A[b, i, j] - 0.5  (centered poly variable, f16)
        xT = (A[bs].transpose(2, 0, 1) - 0.5).astype(np.float16)
        xT = np.ascontiguousarray(xT).reshape(N, BL * N)
        # adj in T layout: [j, (b, i)]
        adjT = adjs.transpose(2, 0, 1).reshape(N, BL * N)
        # coefficient maps: [j, (k, b, i)] with k-major slabs
        cmap = lut[:, adjT]                                # (DEG+1, N, BL*N)
        cmap = np.ascontiguousarray(
            cmap.transpose(1, 0, 2)).reshape(N, (DEG + 1) * BL * N)
        # masks classes 1..4 (select over base class 0)
        m14 = np.empty((N, 4 * BL * N), np.int8)
        for c in range(1, 5):
            m14[:, (c - 1) * BL * N: c * BL * N] = (adjT == (c + 1))

        # hT: [d, (ch, b, j)] = h[b,j,d]  (stationary AND scale source)
        hT = np.empty((128, DCH * BL * 128), BF16)
        for ch in range(DCH):
            for b in range(BL):
                hT[:, (ch * BL + b) * 128:(ch * BL + b + 1) * 128] = (
                    hs[b, :, ch * 128:(ch + 1) * 128].T)
        # ap: [d, (ch, c)] = a_params[ch*128+d, c]
        ap_host = np.empty((128, DCH * 5), np.float32)
        for ch in range(DCH):
            ap_host[:, ch * 5:(ch + 1) * 5] = a_params[ch * 128:(ch + 1) * 128, :]
        # haug: [j, (b, d|1)]
        haug = np.empty((N, BL * (D + 1)), BF16)
        for b in range(BL):
            haug[:, b * (D + 1): b * (D + 1) + D] = hs[b]
            haug[:, b * (D + 1) + D] = 1.0

        in_maps.append({
            "xT": xT, "cm": cmap, "m14": m14, "hT": hT,
            "ap": ap_host, "haug": haug,
        })
    return in_maps


def get_program(inputs: dict):
    nc = _PROG_CACHE.get("prog")
    if nc is None:
        nc = _build_program()
        _split_excess_waits(nc)
        _PROG_CACHE["prog"] = nc
    return nc


# --------------------------------------------------------------------------
# public entry point
# --------------------------------------------------------------------------
def kernel(**inputs: np.ndarray) -> np.ndarray:
    nc = get_program(inputs)
    in_maps = build_in_maps(inputs)

    from concourse.bass_utils import run_bass_kernel_spmd

    res = run_bass_kernel_spmd(nc, in_maps, core_ids=list(range(NCORES)))
    out = np.empty((B, N, D), np.float32)
    for core in range(NCORES):
        o = res.results[core]["out"].astype(np.float32).reshape(N, BL, D)
        out[core * BL: (core + 1) * BL] = o.transpose(1, 0, 2)   # [i,(b,d)]->(b,i,d)
    return out


if __name__ == "__main__":
    rng = np.random.default_rng(0)
    demo = {
        "hidden": rng.standard_normal((B, N, D), dtype=np.float32),
        "A_interval": rng.random((B, N, N), dtype=np.float32),
        "adj": rng.integers(0, 6, (B, N, N)).astype(np.int32),
        "interval_unique": rng.integers(0, 100, (B, N)).astype(np.int32),
        "mask_item": rng.integers(0, 2, (B, N)).astype(np.int32),
        "a_params": (rng.standard_normal((D, 5)) / np.sqrt(D)).astype(np.float32),
        "iw_params": rng.standard_normal((TDIM, 5)).astype(np.float32),
        "te_freq": rng.standard_normal(TDIM).astype(np.float32),
        "te_phase": rng.standard_normal(TDIM).astype(np.float32),
    }
    o = kernel(**demo)
    print("kernel output", o.shape, o.dtype, np.abs(o).max())


# revision 33
# speedup vs baseline: 1.5589x; 1.5589x over previous
"""Trainium2 Bass kernel for nn_LocalAggregator (GNN message passing).

Math (per batch):
    e[i,j,r] = lrelu( h_i . diag(a_r) . h_j  +  sum_t cos(A_ij f_t + p_t) iw[t,r] )
    s[i,j]   = e[i,j,adj_ij-1]  if 1<=adj<=5 else -9e15
    out      = softmax_j(s) @ h

Device strategy (per core, BL=4 of the 32 batches), v2:
  * Everything score-shaped lives TRANSPOSED: sT[j, (b, i)] — legal because
    e1_c = H diag(a_c) H^T is symmetric, and A/adj are transposed host-side.
    This kills the per-batch PE transposes and PSUM->SBUF copies: exp(s)^T
    is directly the stationary operand of the output matmul, and the row
    sums Z come free from a ones-column appended to the moving operand h.
  * e1: bf16 matmuls (1 cyc/row vs fp32's 4) with the 5 classes PACKED into
    the moving operand — hTa5[d,(ch,b,c,i)] host-prescaled by a_c. Per
    (ch,b): one 512-row matmul (classes 0-3 -> psA) + one 128-row (class 4
    -> psB). 16 matmuls / 8 stationary loads instead of 40 / 40.
  * Time-encoding branch g_c(A): host fits a degree-DEG polynomial per
    class (deg-4 max fit err ~5e-3) and GATHERS per-element coefficient
    maps cm_k[j,(b,i)] = C[k, adj-1] — the class select for the polynomial
    happens on the host for free. The device runs ONE Horner chain with
    plain tensor_tensor ops (2(DEG-1) DVE ops total vs 5 per-class chains).
    adj==0 poisoning: cm_0 = -1e5 there (and higher coeffs 0), so after
    lrelu and exp those entries vanish — no separate mask op.
  * e1 class select: scalar-engine copy (class 0 base) + 4 DVE
    copy_predicated with host int8 masks; then one DVE add folds the
    polynomial in, one DVE scalar_tensor_tensor does the leaky relu.
  * exp on the scalar engine -> bf16; out matmul [exp^T stationary] x
    [h | 1] moving gives output AND row-sum Z; 1/Z folds into the
    PSUM->SBUF scaled copy; output DMA'd in bf16.
  * Two walrus version-skew workarounds: the Tile tail drain and any
    instruction may carry at most ONE sync-wait command on this toolchain
    (_patch_tail_drain / _split_excess_waits hoist excess waits onto NoOps).
"""

import os
from contextlib import ExitStack

import numpy as np
import ml_dtypes

BF16 = ml_dtypes.bfloat16

B, N, D, TDIM = 32, 128, 256, 64
NCORES = 8
BL = B // NCORES            # batches per core
ALPHA = 0.2
NEG_INF = -9e15
POISON = -5e4               # adj==0 polynomial poison (f16-finite; exp() == 0)
DEG = 4                     # host-fitted polynomial degree (DEG+1 coefficients)
NWARM = 24                  # PE warm-up matmuls issued during the DMA window
DCH = D // 128              # K-chunks for the e1 contraction
FBJ = BL * N                # 512: free size of (b, i)

_PROG_CACHE: dict = {}
_DRAIN_PATCHED = False


def _patch_tail_drain():
    """Version-skew workaround: the TileContext tail drain accumulates one
    sem-wait per outstanding engine/DMA queue, but this walrus build's Drain
    encoding fits only ONE sync-wait command. Spread the excess waits over
    preceding single-wait NoOps on the same (SP) engine."""
    global _DRAIN_PATCHED
    if _DRAIN_PATCHED:
        return
    import concourse.tile as tile_mod

    def _patched(self, tick_clock, wait_clock):
        nc = self.nc
        drain_inst = nc.sync.drain()
        wait_clock.add_sem_waits(
            drain_inst.ins,
            tile_mod.ScopedClock({None: tick_clock.global_clock}),
        )
        mi = drain_inst.ins
        si = mi.sync_info
        waits = list(si.on_wait) if si is not None and si.on_wait else []
        if len(waits) > 1:
            si.on_wait = waits[:1]
            lst = nc.cur_bb.bb.instructions
            assert lst[-1] is mi, "drain is not the last instruction in block"
            drain_obj = lst.pop()
            for w in waits[1:]:
                nop = nc.sync.nop(nofuse=True)
                nsi = nop.ins.sync_info
                if nsi is None:
                    nop.ins.sync_info = type(si)(on_update=[], on_wait=[w])
                else:
                    nsi.on_wait = [w]
            lst.append(drain_obj)
        nc.all_engine_barrier()
        assert self.sems is not None
        popped = nc._tile_sem_poison_stack.pop()
        assert popped is self._sem_poison
        nc.clear_and_free_semaphores(list(self.sems.allocated().values()))

    tile_mod.TileContext._drain_and_barrier = _patched
    _DRAIN_PATCHED = True


def _split_excess_waits(nc, max_waits: int = 1):
    """This walrus build encodes at most one sync-wait command per
    instruction. Hoist excess waits onto same-engine NoOps inserted
    immediately before the over-subscribed instruction."""
    import concourse.mybir as mybir

    for fn in nc.m.functions:
        for bb in fn.blocks:
            insts = bb.instructions
            i = 0
            while i < len(insts):
                inst = insts[i]
                si = getattr(inst, "sync_info", None)
                waits = list(si.on_wait) if si is not None and si.on_wait else []
                if len(waits) > max_waits:
                    si.on_wait = waits[:max_waits]
                    extra = waits[max_waits:]
                    nops = []
                    for k in range(0, len(extra), max_waits):
                        nops.append(
                            mybir.InstNoOp(
                                name=f"{inst.name}-xw{k}",
                                engine=inst.engine,
                                bass_nofuse=True,
                                sync_info=mybir.SyncInfo(
                                    on_wait=extra[k : k + max_waits], on_update=[]
                                ),
                            )
                        )
                    insts[i:i] = nops
                    i += len(nops)
                i += 1


# --------------------------------------------------------------------------
# host-side parameter preprocessing
# --------------------------------------------------------------------------
def _fit_polys(iw_params: np.ndarray, te_freq: np.ndarray, te_phase: np.ndarray):
    """Least-squares fit of g_c(a) = sum_t iw[t,c] cos(a f_t + p_t), a in [0,1].

    Returns C[k, c] for k=0..DEG, in the CENTERED basis t = a - 0.5
    (keeps coefficients small enough that f16 maps round harmlessly).
    """
    npts = 1024
    x = 0.5 * (1.0 + np.cos(np.pi * (np.arange(npts) + 0.5) / npts))
    f = te_freq.astype(np.float64)
    p = te_phase.astype(np.float64)
    iw = iw_params.astype(np.float64)
    G = np.cos(x[:, None] * f[None, :] + p[None, :]) @ iw      # (npts, 5)
    V = np.vander(x - 0.5, DEG + 1, increasing=True)           # (npts, DEG+1)
    C, *_ = np.linalg.lstsq(V, G, rcond=None)
    return C  # (DEG+1, 5) float64


# --------------------------------------------------------------------------
# Bass program
# --------------------------------------------------------------------------
def _build_program():
    """One-core program; SPMD across 8 cores with per-core input maps.

    All score-shaped tiles are TRANSPOSED: [j, (b, i)] with j (the softmax
    axis) on partitions. The polynomial coefficients arrive as per-element
    maps, so the program itself is independent of the fitted coefficients.
    """
    import concourse.bass as bass
    import concourse.mybir as mybir
    import concourse.tile as tile

    _patch_tail_drain()

    f32 = mybir.dt.float32
    bf16 = mybir.dt.bfloat16
    f16 = mybir.dt.float16
    i8 = mybir.dt.int8
    i16 = mybir.dt.int16
    Alu = mybir.AluOpType
    Act = mybir.ActivationFunctionType

    nc = bass.Bass()

    # DRAM I/O (per-core layouts; host arranges)
    xT_d = nc.dram_tensor("xT", [N, FBJ], f16, kind="ExternalInput")        # (A^T - 0.5) [j,(b,i)]
    cm_d = nc.dram_tensor("cm", [N, (DEG + 1) * FBJ], f16,
                          kind="ExternalInput")                             # coeff maps k=DEG..0
    m14_d = nc.dram_tensor("m14", [N, 4 * FBJ], i8, kind="ExternalInput")   # masks cls 1..4
    hT_d = nc.dram_tensor("hT", [128, DCH * BL * 128], bf16,
                          kind="ExternalInput")                             # [d,(ch,b,j)]
    ap_d = nc.dram_tensor("ap", [128, DCH * 5], f32, kind="ExternalInput")  # a_params [d,(ch,c)]
    haug_d = nc.dram_tensor("haug", [N, BL * (D + 1)], bf16,
                            kind="ExternalInput")                           # [j,(b,d|1)]
    out_d = nc.dram_tensor("out", [N, BL * D], bf16, kind="ExternalOutput") # [i,(b,d)]

    CH_STRIDE = 5 * BL * 128   # 2560: hTa5 per-chunk stride (c-major inside)

    with tile.TileContext(nc) as tc, ExitStack() as ctx:
        io = ctx.enter_context(tc.tile_pool(name="io", bufs=1))
        wrk = ctx.enter_context(tc.tile_pool(name="wrk", bufs=1))

        # ---- PE warm-up scratch: memset issues before any DMA ----
        warm = wrk.tile([128, 128], bf16, tag="warm")
        nc.gpsimd.memset(warm[:], 0)

        # ---- loads, spread across the 3 DMA queues so the gating tensors
        # (ap/hT for the matmul path, xT/cm for the DVE path) land first ----
        ap = io.tile([128, DCH * 5], f32, tag="ap")
        nc.scalar.dma_start(ap[:], ap_d[:])
        xT = io.tile([N, FBJ], f16, tag="xT")
        nc.scalar.dma_start(xT[:], xT_d[:])
        cm = io.tile([N, (DEG + 1) * FBJ], f16, tag="cm")
        nc.gpsimd.dma_start(cm[:, 0:FBJ], cm_d[:, 0:FBJ])
        hT = io.tile([128, DCH * BL * 128], bf16, tag="hT")
        nc.sync.dma_start(hT[:, 0:BL * 128], hT_d[:, 0:BL * 128])
        nc.sync.dma_start(hT[:, BL * 128:], hT_d[:, BL * 128:])
        nc.gpsimd.dma_start(cm[:, FBJ:2 * FBJ], cm_d[:, FBJ:2 * FBJ])
        nc.sync.dma_start(cm[:, 2 * FBJ:3 * FBJ], cm_d[:, 2 * FBJ:3 * FBJ])
        nc.scalar.dma_start(cm[:, 3 * FBJ:], cm_d[:, 3 * FBJ:])
        m14 = io.tile([N, 4 * FBJ], i8, tag="m14")
        nc.gpsimd.dma_start(m14[:], m14_d[:])
        haug = io.tile([N, BL * (D + 1)], bf16, tag="haug")
        nc.gpsimd.dma_start(haug[:], haug_d[:])

        # ---- DVE front half, interleaved with DMA arrivals: scale hT by
        # a_c (bf16 tensor_scalar) around the f16 Horner chain so the DVE
        # never stalls on a single DMA's completion semaphore ----
        # hTa5 layout: [d, (ch, c, b, i)]
        hTa5 = wrk.tile([128, DCH * CH_STRIDE], bf16, tag="hTa5")

        def scale_chunk(ch):
            for c in range(5):
                nc.vector.tensor_scalar(
                    hTa5[:, ch * CH_STRIDE + c * 512:
                         ch * CH_STRIDE + (c + 1) * 512],
                    hT[:, ch * 512:(ch + 1) * 512],
                    ap[:, ch * 5 + c: ch * 5 + c + 1], None, Alu.mult)

        u = wrk.tile([N, FBJ], f16, tag="u")
        scale_chunk(0)
        scale_chunk(1)
        # Horner with per-element coeff maps; cm layout k = DEG ... 0
        nc.vector.tensor_tensor(u[:], cm[:, 0:FBJ], xT[:], Alu.mult)
        for k in range(1, DEG):
            nc.vector.tensor_tensor(u[:], u[:], cm[:, k * FBJ:(k + 1) * FBJ],
                                    Alu.add)
            nc.vector.tensor_tensor(u[:], u[:], xT[:], Alu.mult)
        nc.vector.tensor_tensor(u[:], u[:], cm[:, DEG * FBJ:(DEG + 1) * FBJ],
                                Alu.add)

        # ---- e1 matmuls (classes packed in the moving operand) ----
        s = wrk.tile([N, FBJ], f32, tag="s")
        with tc.tile_pool(name="psum", bufs=1, space="PSUM") as psum:
            psA = psum.tile([N, BL * 512], f32, tag="psA", name="psA")  # (b, c0..3, i)
            psB = psum.tile([N, BL * 128], f32, tag="psB", name="psB")  # (b, c4, i)
            pwarm = psum.tile([128, 128], f32, tag="pwarm", name="pwarm")
            for w in range(NWARM):
                nc.tensor.matmul(pwarm[:], warm[:], warm[:],
                                 start=True, stop=True, skip_group_check=True)
            # hTa5 free layout per chunk is (c, b, i); for batch b the
            # moving operand is the strided 2D view (c x i) at that b.
            hTa5_v = hTa5[:].rearrange("p (ch c b i) -> p ch c b i",
                                       ch=DCH, c=5, b=BL, i=128)
            for ch in range(DCH):
                for b in range(BL):
                    stat = hT[:, (ch * BL + b) * 128:(ch * BL + b + 1) * 128]
                    nc.tensor.matmul(
                        psA[:, b * 512:(b + 1) * 512], stat,
                        hTa5_v[:, ch, 0:4, b, :],
                        start=(ch == 0), stop=(ch == DCH - 1),
                        skip_group_check=True)
                for b in range(BL):
                    stat = hT[:, (ch * BL + b) * 128:(ch * BL + b + 1) * 128]
                    nc.tensor.matmul(
                        psB[:, b * 128:(b + 1) * 128], stat,
                        hTa5_v[:, ch, 4, b, :],
                        start=(ch == 0), stop=(ch == DCH - 1),
                        skip_group_check=True)

            # ---- select e1 by adj class, HALF-SPLIT (b0,b1 | b2,b3) so the
            # whole back end pipelines. The scalar engine (idle here) stages
            # each class map PSUM->SBUF as f16 per half; DVE then applies
            # the predicated class merges per half. ----
            psA_v = psA[:].rearrange("p (b c i) -> p b c i", b=BL, c=4, i=128)
            sf = wrk.tile([N, FBJ], f16, tag="sf")
            sf_v = sf[:].rearrange("p (b i) -> p b i", b=BL, i=128)
            e14 = wrk.tile([N, 4 * FBJ], f16, tag="e14")
            e14_v = e14[:].rearrange("p (c b i) -> p c b i", c=4, b=BL, i=128)
            H = FBJ // 2
            nc.scalar.copy(sf_v, psA_v[:, :, 0, :])       # base: class 0
            for c in range(1, 4):
                nc.scalar.copy(e14_v[:, c - 1, :, :], psA_v[:, :, c, :])
            nc.scalar.copy(e14[:, 3 * FBJ:4 * FBJ], psB[:])

        # per half: class merges, fold polynomial, leaky relu (DVE)
        for h in range(2):
            hs = slice(h * H, (h + 1) * H)
            for c in range(1, 5):
                nc.vector.copy_predicated(
                    sf[:, hs], m14[:, (c - 1) * FBJ + h * H:
                                   (c - 1) * FBJ + (h + 1) * H],
                    e14[:, (c - 1) * FBJ + h * H:(c - 1) * FBJ + (h + 1) * H])
            nc.vector.tensor_tensor(s[:, hs], sf[:, hs], u[:, hs], Alu.add)
            nc.vector.scalar_tensor_tensor(
                s[:, hs], s[:, hs], ALPHA, s[:, hs], Alu.mult, Alu.max)

        # ---- exp -> bf16; out matmul with ones-column Z; scaled copy ----
        ex = wrk.tile([N, FBJ], bf16, tag="ex")
        rz = wrk.tile([N, BL], f32, tag="rz")
        out_sb = wrk.tile([N, BL * D], bf16, tag="out")
        with tc.tile_pool(name="psum2", bufs=1, space="PSUM") as psum2:
            ops = [psum2.tile([N, D + 1], f32, tag=f"op_{b}", name=f"op_{b}")
                   for b in range(BL)]
            for h in range(2):
                hs = slice(h * H, (h + 1) * H)
                nc.scalar.activation(ex[:, hs], s[:, hs], Act.Exp, bias=0.0)
                for b in (2 * h, 2 * h + 1):
                    nc.tensor.matmul(
                        ops[b][:],
                        ex[:, b * 128:(b + 1) * 128],
                        haug[:, b * (D + 1):(b + 1) * (D + 1)],
                    )
                    nc.vector.reciprocal(rz[:, b:b + 1], ops[b][:, D:D + 1])
                    # normalize: alternate DVE / scalar so the tail overlaps
                    if b % 2 == 0:
                        nc.vector.tensor_scalar(
                            out_sb[:, b * D:(b + 1) * D], ops[b][:, 0:D],
                            rz[:, b:b + 1], None, Alu.mult)
                    else:
                        nc.scalar.mul(out_sb[:, b * D:(b + 1) * D],
                                      ops[b][:, 0:D], rz[:, b:b + 1])
                    # DVE-normalized batches issue on the scalar queue and
                    # scalar-normalized ones on sync, so issue never waits
                    # behind the engine that just produced the data
                    eng = nc.scalar if b % 2 == 0 else nc.sync
                    eng.dma_start(out_d[:, b * D:(b + 1) * D],
                                  out_sb[:, b * D:(b + 1) * D])

    return nc


# --------------------------------------------------------------------------
# host-side input maps
# --------------------------------------------------------------------------
def build_in_maps(inputs: dict) -> list:
    hidden = np.ascontiguousarray(inputs["hidden"], dtype=np.float32)   # (B,N,D)
    A = np.ascontiguousarray(inputs["A_interval"], dtype=np.float32)    # (B,N,N)
    adj = np.asarray(inputs["adj"])                                     # (B,N,N) i32
    a_params = np.asarray(inputs["a_params"], dtype=np.float32)         # (D,5)
    Cpoly = _fit_polys(np.asarray(inputs["iw_params"]),
                       np.asarray(inputs["te_freq"]),
                       np.asarray(inputs["te_phase"]))                  # (DEG+1, 5)

    # coefficient lookup tables indexed by adj value 0..5 (0 = poisoned)
    # order in cm: k = DEG, DEG-1, ..., 0
    lut = np.zeros((DEG + 1, 6), np.float16)
    for k in range(DEG + 1):
        lut[DEG - k, 1:] = Cpoly[k, :]          # adj v -> class v-1
    lut[DEG, 0] = POISON                        # c0 for adj==0

    in_maps = []
    for core in range(NCORES):
        bs = slice(core * BL, (core + 1) * BL)
        hs = hidden[bs]                                   # (BL,N,D)
        adjs = adj[bs]
        assert ((adjs >= 1) & (adjs <= 5)).any(axis=2).all(), (
            "row with no valid edge: shift-free softmax unsupported")

        # xT: [j, (b, i)] = A[b, i, j] - 0.5  (centered poly variable, f16)
        xT = (A[bs].transpose(2, 0, 1) - 0.5).astype(np.float16)
        xT = np.ascontiguousarray(xT).reshape(N, BL * N)
        # adj in T layout: [j, (b, i)]
        adjT = adjs.transpose(2, 0, 1).reshape(N, BL * N)
        # coefficient maps: [j, (k, b, i)] with k-major slabs
        cmap = lut[:, adjT]                                # (DEG+1, N, BL*N)
        cmap = np.ascontiguousarray(
            cmap.transpose(1, 0, 2)).reshape(N, (DEG + 1) * BL * N)
        # masks classes 1..4 (select over base class 0)
        m14 = np.empty((N, 4 * BL * N), np.int8)
        for c in range(1, 5):
            m14[:, (c - 1) * BL * N: c * BL * N] = (adjT == (c + 1))

        # hT: [d, (ch, b, j)] = h[b,j,d]  (stationary AND scale source)
        hT = np.empty((128, DCH * BL * 128), BF16)
        for ch in range(DCH):
            for b in range(BL):
                hT[:, (ch * BL + b) * 128:(ch * BL + b + 1) * 128] = (
                    hs[b, :, ch * 128:(ch + 1) * 128].T)
        # ap: [d, (ch, c)] = a_params[ch*128+d, c]
        ap_host = np.empty((128, DCH * 5), np.float32)
        for ch in range(DCH):
            ap_host[:, ch * 5:(ch + 1) * 5] = a_params[ch * 128:(ch + 1) * 128, :]
        # haug: [j, (b, d|1)]
        haug = np.empty((N, BL * (D + 1)), BF16)
        for b in range(BL):
            haug[:, b * (D + 1): b * (D + 1) + D] = hs[b]
            haug[:, b * (D + 1) + D] = 1.0

        in_maps.append({
            "xT": xT, "cm": cmap, "m14": m14, "hT": hT,
            "ap": ap_host, "haug": haug,
        })
    return in_maps


def get_program(inputs: dict):
    nc = _PROG_CACHE.get("prog")
    if nc is None:
        nc = _build_program()
        _split_excess_waits(nc)
        _PROG_CACHE["prog"] = nc
    return nc


# --------------------------------------------------------------------------
# public entry point
# --------------------------------------------------------------------------
def kernel(**inputs: np.ndarray) -> np.ndarray:
    nc = get_program(inputs)
    in_maps = build_in_maps(inputs)

    from concourse.bass_utils import run_bass_kernel_spmd

    res = run_bass_kernel_spmd(nc, in_maps, core_ids=list(range(NCORES)))
    out = np.empty((B, N, D), np.float32)
    for core in range(NCORES):
        o = res.results[core]["out"].astype(np.float32).reshape(N, BL, D)
        out[core * BL: (core + 1) * BL] = o.transpose(1, 0, 2)   # [i,(b,d)]->(b,i,d)
    return out


if __name__ == "__main__":
    rng = np.random.default_rng(0)
    demo = {
        "hidden": rng.standard_normal((B, N, D), dtype=np.float32),
        "A_interval": rng.random((B, N, N), dtype=np.float32),
        "adj": rng.integers(0, 6, (B, N, N)).astype(np.int32),
        "interval_unique": rng.integers(0, 100, (B, N)).astype(np.int32),
        "mask_item": rng.integers(0, 2, (B, N)).astype(np.int32),
        "a_params": (rng.standard_normal((D, 5)) / np.sqrt(D)).astype(np.float32),
        "iw_params": rng.standard_normal((TDIM, 5)).astype(np.float32),
        "te_freq": rng.standard_normal(TDIM).astype(np.float32),
        "te_phase": rng.standard_normal(TDIM).astype(np.float32),
    }
    o = kernel(**demo)
    print("kernel output", o.shape, o.dtype, np.abs(o).max())
